# revision 67
# baseline (speedup 1.0000x reference)
"""Multi-head dot-product attention (causal, f32) on 8 TRN2 NeuronCores.

Sharding (Megatron-style, per sharding hint): batch (2) x head-groups (4 of
4 heads) = 8 cores. Each core computes q/k/v projections for its 4 heads,
causal attention, and the partial output projection Y_c = sum_h O_h @ Wo_h
for its batch. Host sums the 4 partial Y per batch (the "all-reduce").

Kernel strategy (v2): everything the PE touches is bf16 (1 cycle/row at any
moving width; rel-err budget 2e-2 has ~100x headroom over bf16 noise).
Activations live in transposed [feature, token] layout:
    KT_h[d,s]  = sum_e Wk[e,hd] * XkvT[e,s]     (phase B, stationary Wk)
    V[s,hd]    = sum_e XkvT[e,s-blk] * Wv[e,:]  (phase B, stationary XkvT)
    QT_h[d,t]  = sum_e Wq[e,hd] * XqT[e,t]      (fillers, one tile ahead)
    LT[s,t]    = KT_h[:,s-blk].T @ QT_h[:,t]    (one matmul, K=D=128)
    P  = exp(SCALE*LT)      (ACT, bf16 out; diagonal blocks then masked by a
                             0/1 mask multiply on DVE, 2-byte 2x mode)
    OT_h[d,t] += V[s-blk,hd].T @ P              (accumulate over s blocks)
    R[t]: per-s-block partial sums accumulate on DVE (even blocks) and
          GPSIMD (odd blocks); combined + partition_all_reduce (GPSIMD) +
          reciprocal (DVE); OT normalized by a DVE multiply straight out of
          PSUM into bf16 — the whole softmax-denominator chain has ZERO
          tensor-engine instructions.
    Y[t,e]     = sum_h OT_h[:,t-blk].T @ Wo_h   (fillers, one tile behind)

Scheduling: the PE executes in order, and the attention inner loop is
ACT-bound (exp 612ns vs QK+AV 426ns per iteration), so Q-projection and
out-projection matmuls are PACED into the attention stream as fillers, one
per iteration, keeping the PE >97% busy. Diagonal blocks column-slice
QK/exp/AV/esum to the unmasked range (exact causal FLOP count on the PE).
T-tiles are processed in order [2,3,0,1] so the filler-poor heavy tile (3)
gets both YO and Q fillers. Wo is SBUF-resident (loaded once); xq tiles
prefetch two t-tiles ahead on the sync HWDGE queue (ring of 3 so the WAR
wait never blocks SP's sequencer); the scalar queue carries DMA only during
phase B so the ACT sequencer stays clear for exp.

Cost-model timeline: 303.2us vs 369.2us baseline (-17.9%); PE busy ~280us
against a 276.5us exact-causal PE floor at 2.4GHz.
"""
import math
import numpy as np
import ml_dtypes

import concourse.mybir as mybir
import concourse.tile as tile
from concourse import bacc
from concourse import bass_isa
from concourse import bass_utils

f32 = mybir.dt.float32
bf16 = mybir.dt.bfloat16
AF = mybir.ActivationFunctionType

# Problem shape (hardcoded per contract)
B, T, S, E, N, D = 2, 2048, 2048, 2048, 16, 128
N_CORES = 8
HL = 4            # heads per core
P = 128           # partitions

MM_LABELS = {}


def build_nc(T=T, S=S, E=E, HL=HL, TT=512, ST=512):
    """Build the single-core SPMD bass program."""
    NE = E // P           # contraction chunks for projections
    NTT = T // TT         # t tiles
    NST = S // ST         # s tiles in kv phase
    NSB_PER_ST = ST // P  # s blocks per s tile
    NET = E // TT         # e tiles in out projection
    NDIAG = TT // P       # diagonal mask patterns
    MBW = TT + (NDIAG - 1) * P  # wide causal 0/1-mask pattern
    SCALE = 1.0 / math.sqrt(D)

    nc = bacc.Bacc("TRN2", target_bir_lowering=False, debug=False)

    def mm(label, *args, **kw):
        r = nc.tensor.matmul(*args, **kw)
        MM_LABELS[r.ins.name] = label
        return r

    # activations/weights arrive pre-tiled [P, NE, cols] (host transpose is
    # free) so DMA batches 4 e-chunks per dma_start with 2KB+ descriptors
    GR = 4  # e-chunks per DMA granule
    xqT = nc.dram_tensor("xqT", [P, NE, T], bf16, kind="ExternalInput")
    xkvT = nc.dram_tensor("xkvT", [P, NE, S], bf16, kind="ExternalInput")
    wq = nc.dram_tensor("wq", [P, NE, HL * D], bf16, kind="ExternalInput")
    wk = nc.dram_tensor("wk", [P, NE, HL * D], bf16, kind="ExternalInput")
    wv = nc.dram_tensor("wv", [P, NE, HL * D], bf16, kind="ExternalInput")
    wo = nc.dram_tensor("wo", [HL, D, E], bf16, kind="ExternalInput")
    y = nc.dram_tensor("y", [T, E], f32, kind="ExternalOutput")

    with tile.TileContext(nc) as tc:
        with tc.tile_pool(name="persist", bufs=1) as persist:
            kt_all = persist.tile([P, HL, S], bf16)          # K^T [d, h, s]
            v_all = persist.tile([P, S // P, HL * D], bf16)  # V [s-part, blk, hd]
            wo_all = persist.tile([P, HL, E], bf16)          # Wo [d, h, e] resident
            wq_t = persist.tile([P, NE, HL * D], bf16)       # Wq [e-chunk, hd]
            mask01 = persist.tile([P, MBW], bf16)            # wide causal 0/1 mask

            with tc.tile_pool(name="init", bufs=1) as initp:
                scr = initp.tile([P, MBW], f32)
                # W[si, u]: 1 where (u - (NDIAG-1)*P) - si >= 0 else 0
                # diag pattern k is the slice mask01[:, (NDIAG-1-k)*P : +TT]
                nc.gpsimd.memset(scr[:], 1.0)
                nc.gpsimd.affine_select(
                    out=scr[:], in_=scr[:],
                    compare_op=mybir.AluOpType.is_ge,
                    fill=0.0, base=-(NDIAG - 1) * P,
                    pattern=[[1, MBW]], channel_multiplier=-1,
                )
                nc.vector.tensor_copy(mask01[:], scr[:])

            # t-tiles are independent once K/V exist; process the heaviest
            # attention tile (3) early so it can be paced with both YO and Q
            # filler matmuls (tile 3 alone has too few fillers otherwise)
            TILE_ORDER = [2, 3, 0, 1]

            # xq tiles: ring of 3 so the prefetch-2-ahead DMA never WAR-waits;
            # allocate in usage order so ring reuse pairs with long-dead tiles
            xqp_cm = tc.tile_pool(name="xqp", bufs=3)
            xqp = xqp_cm.__enter__()
            xq_tiles = {tt: xqp.tile([P, NE, TT], bf16, tag="xq", name=f"xq{tt}")
                        for tt in TILE_ORDER}

            def load_xq(tt, g=None):
                for g in ([g] if g is not None else range(NE // GR)):
                    nc.sync.dma_start(
                        xq_tiles[tt][:, g * GR:(g + 1) * GR, :],
                        xqT[:, g * GR:(g + 1) * GR, tt * TT:(tt + 1) * TT])

            # ---------------- Phase B: K^T and V for all heads ----------------
            with (
                tc.tile_pool(name="wkv", bufs=1) as wkvp,
                tc.tile_pool(name="xkv", bufs=6) as xp,
                tc.tile_pool(name="pskt", bufs=HL, space="PSUM") as pskt,
                tc.tile_pool(name="psv", bufs=NSB_PER_ST, space="PSUM") as psv,
            ):
                wk_t = wkvp.tile([P, NE, HL * D], bf16)
                wv_t = wkvp.tile([P, NE, HL * D], bf16)

                # weights on the scalar HWDGE queue (idle during phase C),
                # granule-batched; tiny first granules cut the cold-start
                # latency before the first matmul
                GS0 = [(0, 1), (1, 3)] + [(g, GR) for g in range(GR, NE, GR)]
                GS = [(g, GR) for g in range(0, NE, GR)]
                for g0, gn in GS0:
                    nc.scalar.dma_start(wk_t[:, g0:g0 + gn, :], wk[:, g0:g0 + gn, :])
                    nc.scalar.dma_start(wv_t[:, g0:g0 + gn, :], wv[:, g0:g0 + gn, :])
                for g0, gn in GS:
                    nc.scalar.dma_start(wq_t[:, g0:g0 + gn, :], wq[:, g0:g0 + gn, :])
                for h in range(HL):
                    nc.scalar.dma_start(wo_all[:, h, :], wo[h])
                # xq prefetch for the first two processed tiles, interleaved
                # into the last s-tile's load stream so it lands before the
                # B->C transition instead of queueing behind all of xkv
                pending_xq = [(t, g) for t in TILE_ORDER[:2] for g in range(NE // GR)]
                for st in range(NST):
                    psKT = [pskt.tile([P, ST], f32, tag="pskt", name=f"psKT{st}_{h}")
                            for h in range(HL)]
                    psV = [psv.tile([P, HL * D], f32, tag="psv", name=f"psV{st}_{j}")
                           for j in range(NSB_PER_ST)]
                    for g0, gn in (GS0 if st == 0 else GS):
                        xt = xp.tile([P, GR, ST], bf16, tag="xkv",
                                     padded_shape=[P, GR, ST])
                        nc.sync.dma_start(
                            xt[:, 0:gn, :], xkvT[:, g0:g0 + gn, st * ST:(st + 1) * ST])
                        if st == NST - 1:
                            for _ in range(2):
                                if pending_xq:
                                    t_, g_ = pending_xq.pop(0)
                                    load_xq(t_, g_)
                        for i in range(gn):
                            e = g0 + i
                            for h in range(HL):
                                mm("KT", psKT[h][:], wk_t[:, e, h * D:(h + 1) * D],
                                   xt[:, i, :], start=(e == 0), stop=(e == NE - 1))
                            for j in range(NSB_PER_ST):
                                mm("V", psV[j][:], xt[:, i, j * P:(j + 1) * P],
                                   wv_t[:, e, :], start=(e == 0), stop=(e == NE - 1))
                    # psum->sbuf copies: V on DVE. For the last s-tile the K
                    # copies alternate ACT/DVE (and precede V in the DVE
                    # queue) — the first Q matmuls of phase C WAR-wait on
                    # these banks, so the four K copies must drain in ~2
                    # copy-times, not 4. Early s-tiles keep K on DVE (ACT's
                    # sequencer is still clogged by queued weight dma_starts).
                    for h in range(HL):
                        dst = kt_all[:, h, st * ST:(st + 1) * ST]
                        if st == NST - 1 and h % 2 == 0:
                            nc.scalar.activation(dst, psKT[h][:], AF.Copy)
                        else:
                            nc.vector.tensor_copy(dst, psKT[h][:])
                    for j in range(NSB_PER_ST):
                        nc.vector.tensor_copy(v_all[:, st * NSB_PER_ST + j, :],
                                              psV[j][:])
                # any xq granules not absorbed into the st loop
                for t_, g_ in pending_xq:
                    load_xq(t_, g_)

            # ------------- Phase C: Q, attention, output projection -------------
            with (
                tc.tile_pool(name="qt", bufs=2) as qtp,
                tc.tile_pool(name="ep", bufs=4) as epp,
                tc.tile_pool(name="on", bufs=2) as onp,
                tc.tile_pool(name="es", bufs=2) as esp,
                tc.tile_pool(name="rs", bufs=2) as rsp,
                tc.tile_pool(name="ysb", bufs=6) as yp,
                tc.tile_pool(name="psq", bufs=2, space="PSUM") as psq,
                tc.tile_pool(name="psy", bufs=2, space="PSUM") as psy,
                tc.tile_pool(name="psl", bufs=2, space="PSUM") as psl,
                tc.tile_pool(name="pso", bufs=2, space="PSUM") as pso,
            ):
                def make_q_units(tt, qpool=None):
                    """Q projection for tile tt as a list of filler thunks
                    (one thunk = one e-chunk of a 2-head sweep = 2 matmuls).
                    qpool overrides the PSUM pool (the first processed tile
                    runs before any YO work, so psy's banks are free then and
                    lending them kills the sweep-boundary WAR stall).
                    Returns (qt_tile, units)."""
                    qt_t = qtp.tile([P, HL, TT], bf16, tag="qt", name=f"qt{tt}")
                    state = {}

                    def emit(e, sweep):
                        h0, h1 = 2 * sweep, 2 * sweep + 1
                        if e == 0:
                            # sweep 1 on the lent pool (if any): four distinct
                            # banks, so sweep 1 never WAR-waits on sweep 0's
                            # psum->qt copies
                            pool = qpool if (qpool is not None and sweep == 1) else psq
                            tag = "psy" if pool is not psq else "psq"
                            state[sweep] = (
                                pool.tile([P, TT], f32, tag=tag, name=f"psQT{tt}_{h0}"),
                                pool.tile([P, TT], f32, tag=tag, name=f"psQT{tt}_{h1}"),
                            )
                        pa, pb = state[sweep]
                        mm("Q", pa[:], wq_t[:, e, h0 * D:(h0 + 1) * D],
                           xq_tiles[tt][:, e, :], start=(e == 0), stop=(e == NE - 1))
                        mm("Q", pb[:], wq_t[:, e, h1 * D:(h1 + 1) * D],
                           xq_tiles[tt][:, e, :], start=(e == 0), stop=(e == NE - 1))
                        if e == NE - 1:
                            # GPSIMD can't read PSUM, so these stay on DVE;
                            # the four-bank sweep split above keeps their
                            # latency off the PE critical path
                            nc.vector.tensor_copy(qt_t[:, h0, :], pa[:])
                            nc.vector.tensor_copy(qt_t[:, h1, :], pb[:])

                    units = [(lambda e=e, s=sweep: emit(e, s))
                             for sweep in range(HL // 2) for e in range(NE)]
                    return qt_t, units

                def make_yo_units(tt, onorm_t, alt_copy=False, split_last=False,
                                  tail_act=False, alt_from=99):
                    """Out-projection for tile tt as filler thunks
                    (one thunk = one (j, et) group = 4 matmuls + copy + store).
                    alt_copy: alternate the psY->SBUF copy between ACT and DVE
                    (where DVE is saturated). tail_act: last units' copies on
                    ACT so they can't wedge between the final denominator
                    chain's DVE ops. split_last: halve the final group's
                    copy+store to shorten the end-of-kernel drain chain."""
                    units = []
                    for et in range(NET):
                        for j in range(TT // P):
                            def emit(et=et, j=j, n=len(units)):
                                psY = psy.tile([P, TT], f32, tag="psy", name="psY")
                                for h in range(HL):
                                    mm("YO", psY[:], onorm_t[:, h, j * P:(j + 1) * P],
                                       wo_all[:, h, et * TT:(et + 1) * TT],
                                       start=(h == 0), stop=(h == HL - 1))
                                y_t = yp.tile([P, TT], f32, tag="y")
                                eng = (nc.scalar
                                       if ((alt_copy or n >= alt_from) and n % 2)
                                       or (tail_act and n >= 12)
                                       else nc.vector)
                                rows = slice(tt * TT + j * P, tt * TT + (j + 1) * P)
                                if split_last and n == NET * (TT // P) - 1:
                                    # shorten the end-of-kernel chain: two
                                    # halves on parallel engines/queues
                                    c = slice(0, TT // 2)
                                    yc = slice(et * TT, et * TT + TT // 2)
                                    nc.vector.tensor_copy(y_t[:, c], psY[:, c])
                                    nc.sync.dma_start(y[rows, yc], y_t[:, c])
                                    c2 = slice(TT // 2, TT)
                                    yc2 = slice(et * TT + TT // 2, (et + 1) * TT)
                                    nc.scalar.activation(y_t[:, c2], psY[:, c2],
                                                         AF.Copy)
                                    nc.scalar.dma_start(y[rows, yc2], y_t[:, c2])
                                    return
                                if eng is nc.scalar:
                                    nc.scalar.activation(y_t[:], psY[:], AF.Copy)
                                else:
                                    nc.vector.tensor_copy(y_t[:], psY[:])
                                nc.sync.dma_start(
                                    y[rows, et * TT:(et + 1) * TT], y_t[:])
                            units.append(emit)
                    return units

                def attention(tt, qt_t, fillers, lend_q_psum=False):
                    """Causal attention for t-tile tt; paces `fillers` (PE
                    thunks) into the ACT-bound inner loop, one per iteration.
                    lend_q_psum: no Q-projection fillers run during this tile,
                    so psq's two banks are free — rotate psO across both pools
                    to take the denominator chain off the head boundary."""
                    onorm_t = onp.tile([P, HL, TT], bf16, tag="on", name=f"on{tt}")
                    nsb = (tt + 1) * NDIAG
                    diag0 = tt * NDIAG
                    items = [(h, sb) for h in range(HL) for sb in range(nsb)]
                    niter = len(items)
                    psl_of, ep_of = {}, {}

                    def c0_of(h, sb):
                        # diagonal block k: columns < k*P are fully masked —
                        # QK/exp/AV/esum all skip them
                        k = sb - diag0
                        return k * P if k > 0 else 0

                    def qk(i):
                        h, sb = items[i]
                        c0 = c0_of(h, sb)
                        pl = psl.tile([P, TT], f32, tag="psl", name="psL")
                        mm("QK", pl[:, c0:], kt_all[:, h, sb * P:(sb + 1) * P],
                           qt_t[:, h, c0:])
                        psl_of[i] = pl

                    def ex(i):
                        h, sb = items[i]
                        c0 = c0_of(h, sb)
                        ep_t = epp.tile([P, TT], bf16, tag="ep", name="ep")
                        nc.scalar.activation(ep_t[:, c0:], psl_of.pop(i)[:, c0:],
                                             AF.Exp, scale=SCALE)
                        ep_of[i] = ep_t

                    # hold a few fillers back for after the last AV: they keep
                    # the PE busy while the final head's denominator chain
                    # (DVE/GPSIMD) drains, so the next tile / the out-
                    # projection drain doesn't start against a stall
                    reserve = min(8, len(fillers))
                    total = len(fillers) - reserve
                    emitted = 0
                    qk(0)
                    ex(0)
                    psO = esd = esp_t = None
                    for i, (h, sb) in enumerate(items):
                        if i + 1 < niter:
                            qk(i + 1)
                            ex(i + 1)
                        # pace fillers evenly across iterations
                        target = min(total, -(-total * (i + 1) // niter))
                        while emitted < target:
                            fillers[emitted]()
                            emitted += 1
                        ep_t = ep_of.pop(i)
                        k = sb - diag0
                        c0 = c0_of(h, sb)
                        if k >= 0:
                            # only the mixed 128-col block needs masking; its
                            # pattern is the same unit triangle for every k
                            nc.vector.tensor_mul(
                                ep_t[:, c0:c0 + P], ep_t[:, c0:c0 + P],
                                mask01[:, (NDIAG - 1) * P:NDIAG * P])
                        first, last = sb == 0, sb == nsb - 1
                        if first:
                            if lend_q_psum and h % 2:
                                psO = psq.tile([P, TT], f32, tag="psq", name="psO")
                            else:
                                psO = pso.tile([P, TT], f32, tag="pso", name="psO")
                            esd = esp.tile([P, TT], f32, tag="esd", name="esd")
                            esp_t = None
                        mm("AV", psO[:, c0:], v_all[:, sb, h * D:(h + 1) * D],
                           ep_t[:, c0:], start=first, stop=last)
                        # denominator partials: 1/3 of s-blocks accumulate on
                        # GPSIMD, rest on DVE (keeps DVE under the PE
                        # roofline). The final head of the final tile stays
                        # all-DVE so its chain never waits on a GPSIMD
                        # backlog right before the out-projection drain.
                        use_pool = (sb % 3 == 2) and not (
                            tt == TILE_ORDER[-1] and h == HL - 1)
                        if use_pool:
                            if esp_t is None:
                                esp_t = esp.tile([P, TT], f32, tag="esp", name="esp")
                                nc.gpsimd.memset(esp_t[:, 0:c0], 0.0) if c0 else None
                                nc.gpsimd.tensor_copy(esp_t[:, c0:], ep_t[:, c0:])
                            else:
                                nc.gpsimd.tensor_add(esp_t[:, c0:], esp_t[:, c0:],
                                                     ep_t[:, c0:])
                        else:
                            if first:
                                nc.vector.tensor_copy(esd[:], ep_t[:])
                            else:
                                nc.vector.tensor_add(esd[:, c0:], esd[:, c0:],
                                                     ep_t[:, c0:])
                        if last:
                            # softmax denominator + normalization: no PE ops
                            if esp_t is not None:
                                est = esp.tile([P, TT], f32, tag="est", name="est")
                                nc.vector.tensor_add(est[:], esd[:], esp_t[:])
                            else:
                                est = esd
                            rsum = rsp.tile([P, TT], f32, tag="rs", name="rsum")
                            nc.gpsimd.partition_all_reduce(
                                rsum[:], est[:], channels=P,
                                reduce_op=bass_isa.ReduceOp.add)
                            rrec = rsp.tile([P, TT], f32, tag="rr", name="rrec")
                            nc.vector.reciprocal(rrec[:], rsum[:])
                            nc.vector.tensor_mul(onorm_t[:, h, :], psO[:], rrec[:])
                    while emitted < len(fillers):
                        fillers[emitted]()
                        emitted += 1
                    return onorm_t

                # phase C driver: Q for the first processed tile direct, then
                # tiles in TILE_ORDER with paced fillers
                qt_cur, q0_units = make_q_units(TILE_ORDER[0], qpool=psy)
                for u in q0_units:
                    u()
                on_prev, prev_tt = None, None
                carried = []  # YO units deferred from a filler-rich tile to
                              # the filler-poor last tile
                for idx, tt in enumerate(TILE_ORDER):
                    if idx + 2 < NTT:
                        load_xq(TILE_ORDER[idx + 2])
                    if idx + 1 < NTT:
                        qt_next, q_units = make_q_units(
                            TILE_ORDER[idx + 1], qpool=psy if idx == 0 else None)
                    else:
                        qt_next, q_units = None, []
                    if on_prev is not None:
                        if idx == 1:
                            # tile 3 has filler surplus; carry half its YO
                            # units (copies alternating ACT/DVE, since they
                            # will run in the DVE-tight last tile)
                            yo_all = make_yo_units(prev_tt, on_prev, alt_from=8)
                            yo_units, carried = yo_all[:8], yo_all[8:]
                        else:
                            yo_units = make_yo_units(prev_tt, on_prev,
                                                     alt_copy=(idx == NTT - 1),
                                                     tail_act=(idx == NTT - 1))
                    else:
                        yo_units = []
                    # order: Q sweep 0, a few YO groups (covers the DVE
                    # latency of the sweep-0 psum->qt copies before sweep 1's
                    # bank reuse), Q sweep 1, remaining YO; carried units from
                    # the surplus tile lead the last tile's list
                    fillers = (q_units[:NE] + yo_units[:2 * NDIAG]
                               + q_units[NE:] + yo_units[2 * NDIAG:])
                    if idx == NTT - 1:
                        fillers = carried + fillers
                    on_prev = attention(tt, qt_cur, fillers,
                                        lend_q_psum=(idx == NTT - 1))
                    qt_cur, prev_tt = qt_next, tt
                # drain: out projection for the last processed tile
                for u in make_yo_units(prev_tt, on_prev, alt_copy=True,
                                       split_last=True):
                    u()
            xqp_cm.__exit__(None, None, None)

    nc.compile()
    return nc


_NC_CACHE = {}


def _get_nc(key=(T, S, E, HL)):
    if key not in _NC_CACHE:
        _NC_CACHE[key] = build_nc(T=key[0], S=key[1], E=key[2], HL=key[3])
    return _NC_CACHE[key]


def _bf16(x):
    return np.asarray(x, dtype=np.float32).astype(ml_dtypes.bfloat16)


def _tile_pne(xT):
    """[E, cols] -> [P, NE, cols] so one DMA loads several 128-row e-chunks."""
    E_, cols = xT.shape
    return np.ascontiguousarray(
        xT.reshape(E_ // 128, 128, cols).transpose(1, 0, 2))


def kernel(inputs_q, inputs_kv, Wq, Wk, Wv, Wo):
    inputs_q = np.asarray(inputs_q, dtype=np.float32)
    inputs_kv = np.asarray(inputs_kv, dtype=np.float32)
    Wq = np.asarray(Wq, dtype=np.float32)
    Wk = np.asarray(Wk, dtype=np.float32)
    Wv = np.asarray(Wv, dtype=np.float32)
    Wo = np.asarray(Wo, dtype=np.float32)

    nc = _get_nc()

    # shard: core c -> batch c//4, heads [ (c%4)*4, +4 )
    xqT_b = [_tile_pne(_bf16(inputs_q[b].T)) for b in range(B)]
    xkvT_b = [_tile_pne(_bf16(inputs_kv[b].T)) for b in range(B)]
    in_maps = []
    for c in range(N_CORES):
        b, g = divmod(c, N_CORES // B)
        h0 = g * HL
        in_maps.append({
            "xqT": xqT_b[b],
            "xkvT": xkvT_b[b],
            "wq": _tile_pne(_bf16(Wq[:, h0:h0 + HL, :].reshape(E, HL * D))),
            "wk": _tile_pne(_bf16(Wk[:, h0:h0 + HL, :].reshape(E, HL * D))),
            "wv": _tile_pne(_bf16(Wv[:, h0:h0 + HL, :].reshape(E, HL * D))),
            "wo": _bf16(np.ascontiguousarray(Wo[h0:h0 + HL])),
        })

    res = bass_utils.run_bass_kernel_spmd(nc, in_maps, core_ids=list(range(N_CORES)))

    out = np.zeros((B, T, E), dtype=np.float32)
    for c in range(N_CORES):
        b = c // (N_CORES // B)
        out[b] += res.results[c]["y"]
    return out


# revision 68
# speedup vs baseline: 1.0072x; 1.0072x over previous
"""Multi-head dot-product attention (causal, f32) on 8 TRN2 NeuronCores.

Sharding (Megatron-style, per sharding hint): batch (2) x head-groups (4 of
4 heads) = 8 cores. Each core computes q/k/v projections for its 4 heads,
causal attention, and the partial output projection Y_c = sum_h O_h @ Wo_h
for its batch. Host sums the 4 partial Y per batch (the "all-reduce").

Kernel strategy (v2): everything the PE touches is bf16 (1 cycle/row at any
moving width; rel-err budget 2e-2 has ~100x headroom over bf16 noise).
Activations live in transposed [feature, token] layout:
    KT_h[d,s]  = sum_e Wk[e,hd] * XkvT[e,s]     (phase B, stationary Wk)
    V[s,hd]    = sum_e XkvT[e,s-blk] * Wv[e,:]  (phase B, stationary XkvT)
    QT_h[d,t]  = sum_e Wq[e,hd] * XqT[e,t]      (fillers, one tile ahead)
    LT[s,t]    = KT_h[:,s-blk].T @ QT_h[:,t]    (one matmul, K=D=128)
    P  = exp(SCALE*LT)      (ACT, bf16 out; diagonal blocks then masked by a
                             0/1 mask multiply on DVE, 2-byte 2x mode)
    OT_h[d,t] += V[s-blk,hd].T @ P              (accumulate over s blocks)
    R[t]: per-s-block partial sums accumulate on DVE (even blocks) and
          GPSIMD (odd blocks); combined + partition_all_reduce (GPSIMD) +
          reciprocal (DVE); OT normalized by a DVE multiply straight out of
          PSUM into bf16 — the whole softmax-denominator chain has ZERO
          tensor-engine instructions.
    Y[t,e]     = sum_h OT_h[:,t-blk].T @ Wo_h   (fillers, one tile behind)

Scheduling: the PE executes in order, and the attention inner loop is
ACT-bound (exp 612ns vs QK+AV 426ns per iteration), so Q-projection and
out-projection matmuls are PACED into the attention stream as fillers, one
per iteration, keeping the PE >97% busy. Diagonal blocks column-slice
QK/exp/AV/esum to the unmasked range (exact causal FLOP count on the PE).
T-tiles are processed in order [2,3,0,1] so the filler-poor heavy tile (3)
gets both YO and Q fillers. Wo is SBUF-resident (loaded once); xq tiles
prefetch two t-tiles ahead on the sync HWDGE queue (ring of 3 so the WAR
wait never blocks SP's sequencer); the scalar queue carries DMA only during
phase B so the ACT sequencer stays clear for exp.

Cost-model timeline: 303.2us vs 369.2us baseline (-17.9%); PE busy ~280us
against a 276.5us exact-causal PE floor at 2.4GHz.
"""
import math
import numpy as np
import ml_dtypes

import concourse.mybir as mybir
import concourse.tile as tile
from concourse import bacc
from concourse import bass_isa
from concourse import bass_utils

f32 = mybir.dt.float32
bf16 = mybir.dt.bfloat16
AF = mybir.ActivationFunctionType

# Problem shape (hardcoded per contract)
B, T, S, E, N, D = 2, 2048, 2048, 2048, 16, 128
N_CORES = 8
HL = 4            # heads per core
P = 128           # partitions

MM_LABELS = {}


def build_nc(T=T, S=S, E=E, HL=HL, TT=512, ST=512):
    """Build the single-core SPMD bass program."""
    NE = E // P           # contraction chunks for projections
    NTT = T // TT         # t tiles
    NST = S // ST         # s tiles in kv phase
    NSB_PER_ST = ST // P  # s blocks per s tile
    NET = E // TT         # e tiles in out projection
    NDIAG = TT // P       # diagonal mask patterns
    MBW = TT + (NDIAG - 1) * P  # wide causal 0/1-mask pattern
    SCALE = 1.0 / math.sqrt(D)

    nc = bacc.Bacc("TRN2", target_bir_lowering=False, debug=False)

    def mm(label, *args, **kw):
        r = nc.tensor.matmul(*args, **kw)
        MM_LABELS[r.ins.name] = label
        return r

    # activations/weights arrive pre-tiled [P, NE, cols] (host transpose is
    # free) so DMA batches 4 e-chunks per dma_start with 2KB+ descriptors
    GR = 4  # e-chunks per DMA granule
    xqT = nc.dram_tensor("xqT", [P, NE, T], bf16, kind="ExternalInput")
    xkvT = nc.dram_tensor("xkvT", [P, NE, S], bf16, kind="ExternalInput")
    wq = nc.dram_tensor("wq", [P, NE, HL * D], bf16, kind="ExternalInput")
    wk = nc.dram_tensor("wk", [P, NE, HL * D], bf16, kind="ExternalInput")
    wv = nc.dram_tensor("wv", [P, NE, HL * D], bf16, kind="ExternalInput")
    wo = nc.dram_tensor("wo", [HL, D, E], bf16, kind="ExternalInput")
    y = nc.dram_tensor("y", [T, E], f32, kind="ExternalOutput")

    with tile.TileContext(nc) as tc:
        with tc.tile_pool(name="persist", bufs=1) as persist:
            kt_all = persist.tile([P, HL, S], bf16)          # K^T [d, h, s]
            v_all = persist.tile([P, S // P, HL * D], bf16)  # V [s-part, blk, hd]
            wo_all = persist.tile([P, HL, E], bf16)          # Wo [d, h, e] resident
            wq_t = persist.tile([P, NE, HL * D], bf16)       # Wq [e-chunk, hd]
            mask01 = persist.tile([P, MBW], bf16)            # wide causal 0/1 mask

            with tc.tile_pool(name="init", bufs=1) as initp:
                scr = initp.tile([P, MBW], f32)
                # W[si, u]: 1 where (u - (NDIAG-1)*P) - si >= 0 else 0
                # diag pattern k is the slice mask01[:, (NDIAG-1-k)*P : +TT]
                nc.gpsimd.memset(scr[:], 1.0)
                nc.gpsimd.affine_select(
                    out=scr[:], in_=scr[:],
                    compare_op=mybir.AluOpType.is_ge,
                    fill=0.0, base=-(NDIAG - 1) * P,
                    pattern=[[1, MBW]], channel_multiplier=-1,
                )
                nc.vector.tensor_copy(mask01[:], scr[:])

            # t-tiles are independent once K/V exist; process the heaviest
            # attention tile (3) early so it can be paced with both YO and Q
            # filler matmuls (tile 3 alone has too few fillers otherwise)
            TILE_ORDER = [2, 3, 0, 1]

            # xq tiles: ring of 3 so the prefetch-2-ahead DMA never WAR-waits;
            # allocate in usage order so ring reuse pairs with long-dead tiles
            xqp_cm = tc.tile_pool(name="xqp", bufs=3)
            xqp = xqp_cm.__enter__()
            xq_tiles = {tt: xqp.tile([P, NE, TT], bf16, tag="xq", name=f"xq{tt}")
                        for tt in TILE_ORDER}

            def load_xq(tt, g=None):
                for g in ([g] if g is not None else range(NE // GR)):
                    nc.sync.dma_start(
                        xq_tiles[tt][:, g * GR:(g + 1) * GR, :],
                        xqT[:, g * GR:(g + 1) * GR, tt * TT:(tt + 1) * TT])

            # ---------------- Phase B: K^T and V for all heads ----------------
            with (
                tc.tile_pool(name="wkv", bufs=1) as wkvp,
                tc.tile_pool(name="xkv", bufs=6) as xp,
                tc.tile_pool(name="pskt", bufs=HL, space="PSUM") as pskt,
                tc.tile_pool(name="psv", bufs=NSB_PER_ST, space="PSUM") as psv,
            ):
                wk_t = wkvp.tile([P, NE, HL * D], bf16)
                wv_t = wkvp.tile([P, NE, HL * D], bf16)

                # weights on the scalar HWDGE queue (idle during phase C),
                # granule-batched; tiny first granules cut the cold-start
                # latency before the first matmul
                GS0 = [(0, 1), (1, 3)] + [(g, GR) for g in range(GR, NE, GR)]
                GS = [(g, GR) for g in range(0, NE, GR)]
                for g0, gn in GS0:
                    nc.scalar.dma_start(wk_t[:, g0:g0 + gn, :], wk[:, g0:g0 + gn, :])
                    nc.scalar.dma_start(wv_t[:, g0:g0 + gn, :], wv[:, g0:g0 + gn, :])
                for g0, gn in GS:
                    nc.scalar.dma_start(wq_t[:, g0:g0 + gn, :], wq[:, g0:g0 + gn, :])
                for h in range(HL):
                    nc.scalar.dma_start(wo_all[:, h, :], wo[h])
                # xq prefetch for the first two processed tiles, interleaved
                # into the last s-tile's load stream so it lands before the
                # B->C transition instead of queueing behind all of xkv
                pending_xq = [(t, g) for t in TILE_ORDER[:2] for g in range(NE // GR)]
                for st in range(NST):
                    psKT = [pskt.tile([P, ST], f32, tag="pskt", name=f"psKT{st}_{h}")
                            for h in range(HL)]
                    psV = [psv.tile([P, HL * D], f32, tag="psv", name=f"psV{st}_{j}")
                           for j in range(NSB_PER_ST)]
                    for g0, gn in (GS0 if st == 0 else GS):
                        xt = xp.tile([P, GR, ST], bf16, tag="xkv",
                                     padded_shape=[P, GR, ST])
                        nc.sync.dma_start(
                            xt[:, 0:gn, :], xkvT[:, g0:g0 + gn, st * ST:(st + 1) * ST])
                        if st == NST - 1:
                            for _ in range(2):
                                if pending_xq:
                                    t_, g_ = pending_xq.pop(0)
                                    load_xq(t_, g_)
                        for i in range(gn):
                            e = g0 + i
                            for h in range(HL):
                                mm("KT", psKT[h][:], wk_t[:, e, h * D:(h + 1) * D],
                                   xt[:, i, :], start=(e == 0), stop=(e == NE - 1))
                            for j in range(NSB_PER_ST):
                                mm("V", psV[j][:], xt[:, i, j * P:(j + 1) * P],
                                   wv_t[:, e, :], start=(e == 0), stop=(e == NE - 1))
                    # psum->sbuf copies: V on DVE. For the last s-tile the K
                    # copies alternate ACT/DVE (and precede V in the DVE
                    # queue) — the first Q matmuls of phase C WAR-wait on
                    # these banks, so the four K copies must drain in ~2
                    # copy-times, not 4. Early s-tiles keep K on DVE (ACT's
                    # sequencer is still clogged by queued weight dma_starts).
                    for h in range(HL):
                        dst = kt_all[:, h, st * ST:(st + 1) * ST]
                        if st == NST - 1 and h % 2 == 0:
                            nc.scalar.activation(dst, psKT[h][:], AF.Copy)
                        else:
                            nc.vector.tensor_copy(dst, psKT[h][:])
                    for j in range(NSB_PER_ST):
                        nc.vector.tensor_copy(v_all[:, st * NSB_PER_ST + j, :],
                                              psV[j][:])
                # any xq granules not absorbed into the st loop
                for t_, g_ in pending_xq:
                    load_xq(t_, g_)

            # ------------- Phase C: Q, attention, output projection -------------
            with (
                tc.tile_pool(name="qt", bufs=2) as qtp,
                tc.tile_pool(name="ep", bufs=4) as epp,
                tc.tile_pool(name="on", bufs=2) as onp,
                tc.tile_pool(name="es", bufs=2) as esp,
                tc.tile_pool(name="rs", bufs=2) as rsp,
                tc.tile_pool(name="ysb", bufs=6) as yp,
                tc.tile_pool(name="psq", bufs=2, space="PSUM") as psq,
                tc.tile_pool(name="psy", bufs=2, space="PSUM") as psy,
                tc.tile_pool(name="psl", bufs=2, space="PSUM") as psl,
                tc.tile_pool(name="pso", bufs=2, space="PSUM") as pso,
            ):
                def make_q_units(tt, qpool=None):
                    """Q projection for tile tt as a list of filler thunks
                    (one thunk = one e-chunk of a 2-head sweep = 2 matmuls).
                    qpool overrides the PSUM pool (the first processed tile
                    runs before any YO work, so psy's banks are free then and
                    lending them kills the sweep-boundary WAR stall).
                    Returns (qt_tile, units)."""
                    qt_t = qtp.tile([P, HL, TT], bf16, tag="qt", name=f"qt{tt}")
                    state = {}

                    def emit(e, sweep):
                        h0, h1 = 2 * sweep, 2 * sweep + 1
                        if e == 0:
                            # sweep 1 on the lent pool (if any): four distinct
                            # banks, so sweep 1 never WAR-waits on sweep 0's
                            # psum->qt copies
                            pool = qpool if (qpool is not None and sweep == 1) else psq
                            tag = "psy" if pool is not psq else "psq"
                            state[sweep] = (
                                pool.tile([P, TT], f32, tag=tag, name=f"psQT{tt}_{h0}"),
                                pool.tile([P, TT], f32, tag=tag, name=f"psQT{tt}_{h1}"),
                            )
                        pa, pb = state[sweep]
                        mm("Q", pa[:], wq_t[:, e, h0 * D:(h0 + 1) * D],
                           xq_tiles[tt][:, e, :], start=(e == 0), stop=(e == NE - 1))
                        mm("Q", pb[:], wq_t[:, e, h1 * D:(h1 + 1) * D],
                           xq_tiles[tt][:, e, :], start=(e == 0), stop=(e == NE - 1))
                        if e == NE - 1:
                            # GPSIMD can't read PSUM, so these stay on DVE;
                            # the four-bank sweep split above keeps their
                            # latency off the PE critical path
                            nc.vector.tensor_copy(qt_t[:, h0, :], pa[:])
                            nc.vector.tensor_copy(qt_t[:, h1, :], pb[:])

                    units = [(lambda e=e, s=sweep: emit(e, s))
                             for sweep in range(HL // 2) for e in range(NE)]
                    return qt_t, units

                def make_yo_units(tt, onorm_t, alt_copy=False, split_last=False,
                                  tail_act=False, alt_from=99):
                    """Out-projection for tile tt as filler thunks
                    (one thunk = one (j, et) group = 4 matmuls + copy + store).
                    alt_copy: alternate the psY->SBUF copy between ACT and DVE
                    (where DVE is saturated). tail_act: last units' copies on
                    ACT so they can't wedge between the final denominator
                    chain's DVE ops. split_last: halve the final group's
                    copy+store to shorten the end-of-kernel drain chain."""
                    units = []
                    for et in range(NET):
                        for j in range(TT // P):
                            def emit(et=et, j=j, n=len(units)):
                                psY = psy.tile([P, TT], f32, tag="psy", name="psY")
                                for h in range(HL):
                                    mm("YO", psY[:], onorm_t[:, h, j * P:(j + 1) * P],
                                       wo_all[:, h, et * TT:(et + 1) * TT],
                                       start=(h == 0), stop=(h == HL - 1))
                                y_t = yp.tile([P, TT], f32, tag="y")
                                eng = (nc.scalar
                                       if ((alt_copy or n >= alt_from) and n % 2)
                                       or (tail_act and n >= 12)
                                       else nc.vector)
                                rows = slice(tt * TT + j * P, tt * TT + (j + 1) * P)
                                if split_last and n == NET * (TT // P) - 1:
                                    # shorten the end-of-kernel chain: two
                                    # halves on parallel engines/queues
                                    c = slice(0, TT // 2)
                                    yc = slice(et * TT, et * TT + TT // 2)
                                    nc.vector.tensor_copy(y_t[:, c], psY[:, c])
                                    nc.sync.dma_start(y[rows, yc], y_t[:, c])
                                    c2 = slice(TT // 2, TT)
                                    yc2 = slice(et * TT + TT // 2, (et + 1) * TT)
                                    nc.scalar.activation(y_t[:, c2], psY[:, c2],
                                                         AF.Copy)
                                    nc.scalar.dma_start(y[rows, yc2], y_t[:, c2])
                                    return
                                if eng is nc.scalar:
                                    nc.scalar.activation(y_t[:], psY[:], AF.Copy)
                                else:
                                    nc.vector.tensor_copy(y_t[:], psY[:])
                                nc.sync.dma_start(
                                    y[rows, et * TT:(et + 1) * TT], y_t[:])
                            units.append(emit)
                    return units

                def attention(tt, qt_t, fillers, lend_q_psum=False):
                    """Causal attention for t-tile tt; paces `fillers` (PE
                    thunks) into the ACT-bound inner loop, one per iteration.
                    lend_q_psum: no Q-projection fillers run during this tile,
                    so psq's two banks are free — rotate psO across both pools
                    to take the denominator chain off the head boundary."""
                    onorm_t = onp.tile([P, HL, TT], bf16, tag="on", name=f"on{tt}")
                    nsb = (tt + 1) * NDIAG
                    diag0 = tt * NDIAG
                    items = [(h, sb) for h in range(HL) for sb in range(nsb)]
                    niter = len(items)
                    psl_of, ep_of = {}, {}

                    def c0_of(h, sb):
                        # diagonal block k: columns < k*P are fully masked —
                        # QK/exp/AV/esum all skip them
                        k = sb - diag0
                        return k * P if k > 0 else 0

                    def qk(i):
                        h, sb = items[i]
                        c0 = c0_of(h, sb)
                        pl = psl.tile([P, TT], f32, tag="psl", name="psL")
                        mm("QK", pl[:, c0:], kt_all[:, h, sb * P:(sb + 1) * P],
                           qt_t[:, h, c0:])
                        psl_of[i] = pl

                    def ex(i):
                        h, sb = items[i]
                        c0 = c0_of(h, sb)
                        ep_t = epp.tile([P, TT], bf16, tag="ep", name="ep")
                        nc.scalar.activation(ep_t[:, c0:], psl_of.pop(i)[:, c0:],
                                             AF.Exp, scale=SCALE)
                        ep_of[i] = ep_t

                    # hold a few fillers back for after the last AV: they keep
                    # the PE busy while the final head's denominator chain
                    # (DVE/GPSIMD) drains, so the next tile / the out-
                    # projection drain doesn't start against a stall
                    reserve = min(8, len(fillers))
                    total = len(fillers) - reserve
                    emitted = 0
                    qk(0)
                    ex(0)
                    psO = esd = esp_t = None
                    for i, (h, sb) in enumerate(items):
                        if i + 1 < niter:
                            qk(i + 1)
                            ex(i + 1)
                        # pace fillers evenly across iterations
                        target = min(total, -(-total * (i + 1) // niter))
                        while emitted < target:
                            fillers[emitted]()
                            emitted += 1
                        ep_t = ep_of.pop(i)
                        k = sb - diag0
                        c0 = c0_of(h, sb)
                        if k >= 0:
                            # only the mixed 128-col block needs masking; its
                            # pattern is the same unit triangle for every k
                            nc.vector.tensor_mul(
                                ep_t[:, c0:c0 + P], ep_t[:, c0:c0 + P],
                                mask01[:, (NDIAG - 1) * P:NDIAG * P])
                        first, last = sb == 0, sb == nsb - 1
                        if first:
                            if lend_q_psum and h % 2:
                                psO = psq.tile([P, TT], f32, tag="psq", name="psO")
                            else:
                                psO = pso.tile([P, TT], f32, tag="pso", name="psO")
                            esd = esp.tile([P, TT], f32, tag="esd", name="esd")
                            esp_t = None
                        mm("AV", psO[:, c0:], v_all[:, sb, h * D:(h + 1) * D],
                           ep_t[:, c0:], start=first, stop=last)
                        # denominator partials: 1/3 of s-blocks accumulate on
                        # GPSIMD, rest on DVE (keeps DVE under the PE
                        # roofline). The final head of the final tile stays
                        # all-DVE so its chain never waits on a GPSIMD
                        # backlog right before the out-projection drain.
                        use_pool = (sb % 3 == 2) and not (
                            tt == TILE_ORDER[-1] and h == HL - 1)
                        if use_pool:
                            if esp_t is None:
                                esp_t = esp.tile([P, TT], f32, tag="esp", name="esp")
                                nc.gpsimd.memset(esp_t[:, 0:c0], 0.0) if c0 else None
                                nc.gpsimd.tensor_copy(esp_t[:, c0:], ep_t[:, c0:])
                            else:
                                nc.gpsimd.tensor_add(esp_t[:, c0:], esp_t[:, c0:],
                                                     ep_t[:, c0:])
                        else:
                            if first:
                                nc.vector.tensor_copy(esd[:], ep_t[:])
                            else:
                                nc.vector.tensor_add(esd[:, c0:], esd[:, c0:],
                                                     ep_t[:, c0:])
                        if last:
                            # softmax denominator + normalization: no PE ops
                            if esp_t is not None:
                                est = esp.tile([P, TT], f32, tag="est", name="est")
                                nc.vector.tensor_add(est[:], esd[:], esp_t[:])
                            else:
                                est = esd
                            rsum = rsp.tile([P, TT], f32, tag="rs", name="rsum")
                            nc.gpsimd.partition_all_reduce(
                                rsum[:], est[:], channels=P,
                                reduce_op=bass_isa.ReduceOp.add)
                            rrec = rsp.tile([P, TT], f32, tag="rr", name="rrec")
                            nc.vector.reciprocal(rrec[:], rsum[:])
                            nc.vector.tensor_mul(onorm_t[:, h, :], psO[:], rrec[:])
                    while emitted < len(fillers):
                        fillers[emitted]()
                        emitted += 1
                    return onorm_t

                # phase C driver: Q for the first processed tile direct, then
                # tiles in TILE_ORDER with paced fillers
                qt_cur, q0_units = make_q_units(TILE_ORDER[0], qpool=psy)
                for u in q0_units:
                    u()
                on_prev, prev_tt = None, None
                carried = []  # YO units deferred from a filler-rich tile to
                              # the filler-poor last tile
                for idx, tt in enumerate(TILE_ORDER):
                    if idx + 2 < NTT:
                        load_xq(TILE_ORDER[idx + 2])
                    if idx + 1 < NTT:
                        qt_next, q_units = make_q_units(
                            TILE_ORDER[idx + 1], qpool=psy if idx == 0 else None)
                    else:
                        qt_next, q_units = None, []
                    if on_prev is not None:
                        if idx == 1:
                            # tile 3 has filler surplus; carry half its YO
                            # units (copies alternating ACT/DVE, since they
                            # will run in the DVE-tight last tile)
                            yo_all = make_yo_units(prev_tt, on_prev, alt_from=12)
                            yo_units, carried = yo_all[:12], yo_all[12:]
                        else:
                            yo_units = make_yo_units(prev_tt, on_prev,
                                                     alt_copy=(idx == NTT - 1),
                                                     tail_act=(idx == NTT - 1))
                    else:
                        yo_units = []
                    # order: Q sweep 0, a few YO groups (covers the DVE
                    # latency of the sweep-0 psum->qt copies before sweep 1's
                    # bank reuse), Q sweep 1, remaining YO; carried units from
                    # the surplus tile lead the last tile's list
                    fillers = (q_units[:NE] + yo_units[:2 * NDIAG]
                               + q_units[NE:] + yo_units[2 * NDIAG:])
                    if idx == NTT - 1:
                        fillers = carried + fillers
                    on_prev = attention(tt, qt_cur, fillers,
                                        lend_q_psum=(idx == NTT - 1))
                    qt_cur, prev_tt = qt_next, tt
                # drain: out projection for the last processed tile
                for u in make_yo_units(prev_tt, on_prev, alt_copy=True,
                                       split_last=True):
                    u()
            xqp_cm.__exit__(None, None, None)

    nc.compile()
    return nc


_NC_CACHE = {}


def _get_nc(key=(T, S, E, HL)):
    if key not in _NC_CACHE:
        _NC_CACHE[key] = build_nc(T=key[0], S=key[1], E=key[2], HL=key[3])
    return _NC_CACHE[key]


def _bf16(x):
    return np.asarray(x, dtype=np.float32).astype(ml_dtypes.bfloat16)


def _tile_pne(xT):
    """[E, cols] -> [P, NE, cols] so one DMA loads several 128-row e-chunks."""
    E_, cols = xT.shape
    return np.ascontiguousarray(
        xT.reshape(E_ // 128, 128, cols).transpose(1, 0, 2))


def kernel(inputs_q, inputs_kv, Wq, Wk, Wv, Wo):
    inputs_q = np.asarray(inputs_q, dtype=np.float32)
    inputs_kv = np.asarray(inputs_kv, dtype=np.float32)
    Wq = np.asarray(Wq, dtype=np.float32)
    Wk = np.asarray(Wk, dtype=np.float32)
    Wv = np.asarray(Wv, dtype=np.float32)
    Wo = np.asarray(Wo, dtype=np.float32)

    nc = _get_nc()

    # shard: core c -> batch c//4, heads [ (c%4)*4, +4 )
    xqT_b = [_tile_pne(_bf16(inputs_q[b].T)) for b in range(B)]
    xkvT_b = [_tile_pne(_bf16(inputs_kv[b].T)) for b in range(B)]
    in_maps = []
    for c in range(N_CORES):
        b, g = divmod(c, N_CORES // B)
        h0 = g * HL
        in_maps.append({
            "xqT": xqT_b[b],
            "xkvT": xkvT_b[b],
            "wq": _tile_pne(_bf16(Wq[:, h0:h0 + HL, :].reshape(E, HL * D))),
            "wk": _tile_pne(_bf16(Wk[:, h0:h0 + HL, :].reshape(E, HL * D))),
            "wv": _tile_pne(_bf16(Wv[:, h0:h0 + HL, :].reshape(E, HL * D))),
            "wo": _bf16(np.ascontiguousarray(Wo[h0:h0 + HL])),
        })

    res = bass_utils.run_bass_kernel_spmd(nc, in_maps, core_ids=list(range(N_CORES)))

    out = np.zeros((B, T, E), dtype=np.float32)
    for c in range(N_CORES):
        b = c // (N_CORES // B)
        out[b] += res.results[c]["y"]
    return out


# revision 69
# speedup vs baseline: 1.0142x; 1.0069x over previous
"""Multi-head dot-product attention (causal, f32) on 8 TRN2 NeuronCores.

Sharding (Megatron-style, per sharding hint): batch (2) x head-groups (4 of
4 heads) = 8 cores. Each core computes q/k/v projections for its 4 heads,
causal attention, and the partial output projection Y_c = sum_h O_h @ Wo_h
for its batch. Host sums the 4 partial Y per batch (the "all-reduce").

Kernel strategy (v2): everything the PE touches is bf16 (1 cycle/row at any
moving width; rel-err budget 2e-2 has ~100x headroom over bf16 noise).
Activations live in transposed [feature, token] layout:
    KT_h[d,s]  = sum_e Wk[e,hd] * XkvT[e,s]     (phase B, stationary Wk)
    V[s,hd]    = sum_e XkvT[e,s-blk] * Wv[e,:]  (phase B, stationary XkvT)
    QT_h[d,t]  = sum_e Wq[e,hd] * XqT[e,t]      (fillers, one tile ahead)
    LT[s,t]    = KT_h[:,s-blk].T @ QT_h[:,t]    (one matmul, K=D=128)
    P  = exp(SCALE*LT)      (ACT, bf16 out; diagonal blocks then masked by a
                             0/1 mask multiply on DVE, 2-byte 2x mode)
    OT_h[d,t] += V[s-blk,hd].T @ P              (accumulate over s blocks)
    R[t]: per-s-block partial sums accumulate on DVE (even blocks) and
          GPSIMD (odd blocks); combined + partition_all_reduce (GPSIMD) +
          reciprocal (DVE); OT normalized by a DVE multiply straight out of
          PSUM into bf16 — the whole softmax-denominator chain has ZERO
          tensor-engine instructions.
    Y[t,e]     = sum_h OT_h[:,t-blk].T @ Wo_h   (fillers, one tile behind)

Scheduling: the PE executes in order, and the attention inner loop is
ACT-bound (exp 612ns vs QK+AV 426ns per iteration), so Q-projection and
out-projection matmuls are PACED into the attention stream as fillers, one
per iteration, keeping the PE >97% busy. Diagonal blocks column-slice
QK/exp/AV/esum to the unmasked range (exact causal FLOP count on the PE).
T-tiles are processed in order [2,3,0,1] so the filler-poor heavy tile (3)
gets both YO and Q fillers. Wo is SBUF-resident (loaded once); xq tiles
prefetch two t-tiles ahead on the sync HWDGE queue (ring of 3 so the WAR
wait never blocks SP's sequencer); the scalar queue carries DMA only during
phase B so the ACT sequencer stays clear for exp.

Cost-model timeline: 303.2us vs 369.2us baseline (-17.9%); PE busy ~280us
against a 276.5us exact-causal PE floor at 2.4GHz.
"""
import math
import numpy as np
import ml_dtypes

import concourse.mybir as mybir
import concourse.tile as tile
from concourse import bacc
from concourse import bass_isa
from concourse import bass_utils

f32 = mybir.dt.float32
bf16 = mybir.dt.bfloat16
AF = mybir.ActivationFunctionType

# Problem shape (hardcoded per contract)
B, T, S, E, N, D = 2, 2048, 2048, 2048, 16, 128
N_CORES = 8
HL = 4            # heads per core
P = 128           # partitions

MM_LABELS = {}


def build_nc(T=T, S=S, E=E, HL=HL, TT=512, ST=512):
    """Build the single-core SPMD bass program."""
    NE = E // P           # contraction chunks for projections
    NTT = T // TT         # t tiles
    NST = S // ST         # s tiles in kv phase
    NSB_PER_ST = ST // P  # s blocks per s tile
    NET = E // TT         # e tiles in out projection
    NDIAG = TT // P       # diagonal mask patterns
    MBW = TT + (NDIAG - 1) * P  # wide causal 0/1-mask pattern
    SCALE = 1.0 / math.sqrt(D)

    nc = bacc.Bacc("TRN2", target_bir_lowering=False, debug=False)

    def mm(label, *args, **kw):
        r = nc.tensor.matmul(*args, **kw)
        MM_LABELS[r.ins.name] = label
        return r

    # activations/weights arrive pre-tiled [P, NE, cols] (host transpose is
    # free) so DMA batches 4 e-chunks per dma_start with 2KB+ descriptors
    GR = 4  # e-chunks per DMA granule
    xqT = nc.dram_tensor("xqT", [P, NE, T], bf16, kind="ExternalInput")
    xkvT = nc.dram_tensor("xkvT", [P, NE, S], bf16, kind="ExternalInput")
    wq = nc.dram_tensor("wq", [P, NE, HL * D], bf16, kind="ExternalInput")
    wk = nc.dram_tensor("wk", [P, NE, HL * D], bf16, kind="ExternalInput")
    wv = nc.dram_tensor("wv", [P, NE, HL * D], bf16, kind="ExternalInput")
    wo = nc.dram_tensor("wo", [HL, D, E], bf16, kind="ExternalInput")
    y = nc.dram_tensor("y", [T, E], f32, kind="ExternalOutput")

    with tile.TileContext(nc) as tc:
        with tc.tile_pool(name="persist", bufs=1) as persist:
            kt_all = persist.tile([P, HL, S], bf16)          # K^T [d, h, s]
            v_all = persist.tile([P, S // P, HL * D], bf16)  # V [s-part, blk, hd]
            wo_all = persist.tile([P, HL, E], bf16)          # Wo [d, h, e] resident
            wq_t = persist.tile([P, NE, HL * D], bf16)       # Wq [e-chunk, hd]
            mask01 = persist.tile([P, MBW], bf16)            # wide causal 0/1 mask

            with tc.tile_pool(name="init", bufs=1) as initp:
                scr = initp.tile([P, MBW], f32)
                # W[si, u]: 1 where (u - (NDIAG-1)*P) - si >= 0 else 0
                # diag pattern k is the slice mask01[:, (NDIAG-1-k)*P : +TT]
                nc.gpsimd.memset(scr[:], 1.0)
                nc.gpsimd.affine_select(
                    out=scr[:], in_=scr[:],
                    compare_op=mybir.AluOpType.is_ge,
                    fill=0.0, base=-(NDIAG - 1) * P,
                    pattern=[[1, MBW]], channel_multiplier=-1,
                )
                nc.vector.tensor_copy(mask01[:], scr[:])

            # t-tiles are independent once K/V exist; process the heaviest
            # attention tile (3) early so it can be paced with both YO and Q
            # filler matmuls (tile 3 alone has too few fillers otherwise)
            TILE_ORDER = [2, 3, 0, 1]

            # xq tiles: ring of 3 so the prefetch-2-ahead DMA never WAR-waits;
            # allocate in usage order so ring reuse pairs with long-dead tiles
            xqp_cm = tc.tile_pool(name="xqp", bufs=3)
            xqp = xqp_cm.__enter__()
            xq_tiles = {tt: xqp.tile([P, NE, TT], bf16, tag="xq", name=f"xq{tt}")
                        for tt in TILE_ORDER}

            def load_xq(tt, g=None):
                for g in ([g] if g is not None else range(NE // GR)):
                    nc.sync.dma_start(
                        xq_tiles[tt][:, g * GR:(g + 1) * GR, :],
                        xqT[:, g * GR:(g + 1) * GR, tt * TT:(tt + 1) * TT])

            # ---------------- Phase B: K^T and V for all heads ----------------
            with (
                tc.tile_pool(name="wkv", bufs=1) as wkvp,
                tc.tile_pool(name="xkv", bufs=6) as xp,
                tc.tile_pool(name="pskt", bufs=HL, space="PSUM") as pskt,
                tc.tile_pool(name="psv", bufs=NSB_PER_ST, space="PSUM") as psv,
            ):
                wk_t = wkvp.tile([P, NE, HL * D], bf16)
                wv_t = wkvp.tile([P, NE, HL * D], bf16)

                # weights on the scalar HWDGE queue (idle during phase C),
                # granule-batched; tiny first granules cut the cold-start
                # latency before the first matmul
                GS0 = [(0, 1), (1, 3)] + [(g, GR) for g in range(GR, NE, GR)]
                GS = [(g, GR) for g in range(0, NE, GR)]
                for g0, gn in GS0:
                    nc.scalar.dma_start(wk_t[:, g0:g0 + gn, :], wk[:, g0:g0 + gn, :])
                    nc.scalar.dma_start(wv_t[:, g0:g0 + gn, :], wv[:, g0:g0 + gn, :])
                for g0, gn in GS:
                    nc.scalar.dma_start(wq_t[:, g0:g0 + gn, :], wq[:, g0:g0 + gn, :])
                for h in range(HL):
                    nc.scalar.dma_start(wo_all[:, h, :], wo[h])
                # xq prefetch for the first two processed tiles, interleaved
                # into the last s-tile's load stream so it lands before the
                # B->C transition instead of queueing behind all of xkv
                pending_xq = [(t, g) for t in TILE_ORDER[:2] for g in range(NE // GR)]
                for st in range(NST):
                    psKT = [pskt.tile([P, ST], f32, tag="pskt", name=f"psKT{st}_{h}")
                            for h in range(HL)]
                    psV = [psv.tile([P, HL * D], f32, tag="psv", name=f"psV{st}_{j}")
                           for j in range(NSB_PER_ST)]
                    for g0, gn in (GS0 if st == 0 else GS):
                        xt = xp.tile([P, GR, ST], bf16, tag="xkv",
                                     padded_shape=[P, GR, ST])
                        nc.sync.dma_start(
                            xt[:, 0:gn, :], xkvT[:, g0:g0 + gn, st * ST:(st + 1) * ST])
                        if st == NST - 1:
                            for _ in range(2):
                                if pending_xq:
                                    t_, g_ = pending_xq.pop(0)
                                    load_xq(t_, g_)
                        for i in range(gn):
                            e = g0 + i
                            for h in range(HL):
                                mm("KT", psKT[h][:], wk_t[:, e, h * D:(h + 1) * D],
                                   xt[:, i, :], start=(e == 0), stop=(e == NE - 1))
                            for j in range(NSB_PER_ST):
                                mm("V", psV[j][:], xt[:, i, j * P:(j + 1) * P],
                                   wv_t[:, e, :], start=(e == 0), stop=(e == NE - 1))
                    # psum->sbuf copies: V on DVE. For the last s-tile the K
                    # copies alternate ACT/DVE (and precede V in the DVE
                    # queue) — the first Q matmuls of phase C WAR-wait on
                    # these banks, so the four K copies must drain in ~2
                    # copy-times, not 4. Early s-tiles keep K on DVE (ACT's
                    # sequencer is still clogged by queued weight dma_starts).
                    for h in range(HL):
                        dst = kt_all[:, h, st * ST:(st + 1) * ST]
                        if st == NST - 1 and h % 2 == 0:
                            nc.scalar.activation(dst, psKT[h][:], AF.Copy)
                        else:
                            nc.vector.tensor_copy(dst, psKT[h][:])
                    for j in range(NSB_PER_ST):
                        nc.vector.tensor_copy(v_all[:, st * NSB_PER_ST + j, :],
                                              psV[j][:])
                # any xq granules not absorbed into the st loop
                for t_, g_ in pending_xq:
                    load_xq(t_, g_)

            # ------------- Phase C: Q, attention, output projection -------------
            with (
                tc.tile_pool(name="qt", bufs=2) as qtp,
                tc.tile_pool(name="ep", bufs=4) as epp,
                tc.tile_pool(name="on", bufs=2) as onp,
                tc.tile_pool(name="es", bufs=2) as esp,
                tc.tile_pool(name="rs", bufs=2) as rsp,
                tc.tile_pool(name="ysb", bufs=6) as yp,
                tc.tile_pool(name="psq", bufs=2, space="PSUM") as psq,
                tc.tile_pool(name="psy", bufs=2, space="PSUM") as psy,
                tc.tile_pool(name="psl", bufs=2, space="PSUM") as psl,
                tc.tile_pool(name="pso", bufs=2, space="PSUM") as pso,
            ):
                def make_q_units(tt, qpool=None):
                    """Q projection for tile tt as a list of filler thunks
                    (one thunk = one e-chunk of a 2-head sweep = 2 matmuls).
                    qpool overrides the PSUM pool (the first processed tile
                    runs before any YO work, so psy's banks are free then and
                    lending them kills the sweep-boundary WAR stall).
                    Returns (qt_tile, units)."""
                    qt_t = qtp.tile([P, HL, TT], bf16, tag="qt", name=f"qt{tt}")
                    state = {}

                    def emit(e, sweep):
                        h0, h1 = 2 * sweep, 2 * sweep + 1
                        if e == 0:
                            # sweep 1 on the lent pool (if any): four distinct
                            # banks, so sweep 1 never WAR-waits on sweep 0's
                            # psum->qt copies
                            pool = qpool if (qpool is not None and sweep == 1) else psq
                            tag = "psy" if pool is not psq else "psq"
                            state[sweep] = (
                                pool.tile([P, TT], f32, tag=tag, name=f"psQT{tt}_{h0}"),
                                pool.tile([P, TT], f32, tag=tag, name=f"psQT{tt}_{h1}"),
                            )
                        pa, pb = state[sweep]
                        mm("Q", pa[:], wq_t[:, e, h0 * D:(h0 + 1) * D],
                           xq_tiles[tt][:, e, :], start=(e == 0), stop=(e == NE - 1))
                        mm("Q", pb[:], wq_t[:, e, h1 * D:(h1 + 1) * D],
                           xq_tiles[tt][:, e, :], start=(e == 0), stop=(e == NE - 1))
                        if e == NE - 1:
                            # GPSIMD can't read PSUM, so these stay on DVE;
                            # the four-bank sweep split above keeps their
                            # latency off the PE critical path
                            nc.vector.tensor_copy(qt_t[:, h0, :], pa[:])
                            nc.vector.tensor_copy(qt_t[:, h1, :], pb[:])

                    units = [(lambda e=e, s=sweep: emit(e, s))
                             for sweep in range(HL // 2) for e in range(NE)]
                    return qt_t, units

                def make_yo_units(tt, onorm_t, alt_copy=False, split_last=False,
                                  tail_act=False, alt_from=99):
                    """Out-projection for tile tt as filler thunks
                    (one thunk = one (j, et) group = 4 matmuls + copy + store).
                    alt_copy: alternate the psY->SBUF copy between ACT and DVE
                    (where DVE is saturated). tail_act: last units' copies on
                    ACT so they can't wedge between the final denominator
                    chain's DVE ops. split_last: halve the final group's
                    copy+store to shorten the end-of-kernel drain chain."""
                    units = []
                    for et in range(NET):
                        for j in range(TT // P):
                            def emit(et=et, j=j, n=len(units)):
                                psY = psy.tile([P, TT], f32, tag="psy", name="psY")
                                for h in range(HL):
                                    mm("YO", psY[:], onorm_t[:, h, j * P:(j + 1) * P],
                                       wo_all[:, h, et * TT:(et + 1) * TT],
                                       start=(h == 0), stop=(h == HL - 1))
                                y_t = yp.tile([P, TT], f32, tag="y")
                                eng = (nc.scalar
                                       if ((alt_copy or n >= alt_from) and n % 2)
                                       or (tail_act and n >= 12)
                                       else nc.vector)
                                rows = slice(tt * TT + j * P, tt * TT + (j + 1) * P)
                                if split_last and n == NET * (TT // P) - 1:
                                    # shorten the end-of-kernel chain: two
                                    # halves on parallel engines/queues
                                    c = slice(0, TT // 2)
                                    yc = slice(et * TT, et * TT + TT // 2)
                                    nc.vector.tensor_copy(y_t[:, c], psY[:, c])
                                    nc.sync.dma_start(y[rows, yc], y_t[:, c])
                                    c2 = slice(TT // 2, TT)
                                    yc2 = slice(et * TT + TT // 2, (et + 1) * TT)
                                    nc.scalar.activation(y_t[:, c2], psY[:, c2],
                                                         AF.Copy)
                                    nc.scalar.dma_start(y[rows, yc2], y_t[:, c2])
                                    return
                                if eng is nc.scalar:
                                    nc.scalar.activation(y_t[:], psY[:], AF.Copy)
                                else:
                                    nc.vector.tensor_copy(y_t[:], psY[:])
                                nc.sync.dma_start(
                                    y[rows, et * TT:(et + 1) * TT], y_t[:])
                            units.append(emit)
                    return units

                def attention(tt, qt_t, fillers, lend_q_psum=False):
                    """Causal attention for t-tile tt; paces `fillers` (PE
                    thunks) into the ACT-bound inner loop, one per iteration.
                    lend_q_psum: no Q-projection fillers run during this tile,
                    so psq's two banks are free — rotate psO across both pools
                    to take the denominator chain off the head boundary."""
                    onorm_t = onp.tile([P, HL, TT], bf16, tag="on", name=f"on{tt}")
                    nsb = (tt + 1) * NDIAG
                    diag0 = tt * NDIAG
                    items = [(h, sb) for h in range(HL) for sb in range(nsb)]
                    niter = len(items)
                    psl_of, ep_of = {}, {}

                    def c0_of(h, sb):
                        # diagonal block k: columns < k*P are fully masked —
                        # QK/exp/AV/esum all skip them
                        k = sb - diag0
                        return k * P if k > 0 else 0

                    def qk(i):
                        h, sb = items[i]
                        c0 = c0_of(h, sb)
                        pl = psl.tile([P, TT], f32, tag="psl", name="psL")
                        mm("QK", pl[:, c0:], kt_all[:, h, sb * P:(sb + 1) * P],
                           qt_t[:, h, c0:])
                        psl_of[i] = pl

                    def ex(i):
                        h, sb = items[i]
                        c0 = c0_of(h, sb)
                        ep_t = epp.tile([P, TT], bf16, tag="ep", name="ep")
                        nc.scalar.activation(ep_t[:, c0:], psl_of.pop(i)[:, c0:],
                                             AF.Exp, scale=SCALE)
                        ep_of[i] = ep_t

                    # hold a few fillers back for after the last AV: they keep
                    # the PE busy while the final head's denominator chain
                    # (DVE/GPSIMD) drains, so the next tile / the out-
                    # projection drain doesn't start against a stall
                    reserve = min(8, len(fillers))
                    total = len(fillers) - reserve
                    emitted = 0
                    qk(0)
                    ex(0)
                    psO = esd = esp_t = None
                    for i, (h, sb) in enumerate(items):
                        if i + 1 < niter:
                            qk(i + 1)
                            ex(i + 1)
                        # pace fillers evenly across iterations
                        target = min(total, -(-total * (i + 1) // niter))
                        while emitted < target:
                            fillers[emitted]()
                            emitted += 1
                        ep_t = ep_of.pop(i)
                        k = sb - diag0
                        c0 = c0_of(h, sb)
                        if k >= 0:
                            # only the mixed 128-col block needs masking; its
                            # pattern is the same unit triangle for every k
                            nc.vector.tensor_mul(
                                ep_t[:, c0:c0 + P], ep_t[:, c0:c0 + P],
                                mask01[:, (NDIAG - 1) * P:NDIAG * P])
                        first, last = sb == 0, sb == nsb - 1
                        if first:
                            if lend_q_psum and h % 2:
                                psO = psq.tile([P, TT], f32, tag="psq", name="psO")
                            else:
                                psO = pso.tile([P, TT], f32, tag="pso", name="psO")
                            esd = esp.tile([P, TT], f32, tag="esd", name="esd")
                            esp_t = None
                        mm("AV", psO[:, c0:], v_all[:, sb, h * D:(h + 1) * D],
                           ep_t[:, c0:], start=first, stop=last)
                        # denominator partials: 1/3 of s-blocks accumulate on
                        # GPSIMD, rest on DVE (keeps DVE under the PE
                        # roofline). The final head of the final tile stays
                        # all-DVE so its chain never waits on a GPSIMD
                        # backlog right before the out-projection drain.
                        use_pool = (sb % 3 == 2) and not (
                            tt == TILE_ORDER[-1] and h == HL - 1)
                        if use_pool:
                            if esp_t is None:
                                esp_t = esp.tile([P, TT], f32, tag="esp", name="esp")
                                nc.gpsimd.memset(esp_t[:, 0:c0], 0.0) if c0 else None
                                nc.gpsimd.tensor_copy(esp_t[:, c0:], ep_t[:, c0:])
                            else:
                                nc.gpsimd.tensor_add(esp_t[:, c0:], esp_t[:, c0:],
                                                     ep_t[:, c0:])
                        else:
                            if first:
                                nc.vector.tensor_copy(esd[:], ep_t[:])
                            else:
                                nc.vector.tensor_add(esd[:, c0:], esd[:, c0:],
                                                     ep_t[:, c0:])
                        if last:
                            # softmax denominator + normalization: no PE ops
                            if esp_t is not None:
                                est = esp.tile([P, TT], f32, tag="est", name="est")
                                nc.vector.tensor_add(est[:], esd[:], esp_t[:])
                            else:
                                est = esd
                            rsum = rsp.tile([P, TT], f32, tag="rs", name="rsum")
                            nc.gpsimd.partition_all_reduce(
                                rsum[:], est[:], channels=P,
                                reduce_op=bass_isa.ReduceOp.add)
                            rrec = rsp.tile([P, TT], f32, tag="rr", name="rrec")
                            nc.vector.reciprocal(rrec[:], rsum[:])
                            nc.vector.tensor_mul(onorm_t[:, h, :], psO[:], rrec[:])
                    while emitted < len(fillers):
                        fillers[emitted]()
                        emitted += 1
                    return onorm_t

                # phase C driver: Q for the first processed tile direct, then
                # tiles in TILE_ORDER with paced fillers
                qt_cur, q0_units = make_q_units(TILE_ORDER[0], qpool=psy)
                for u in q0_units:
                    u()
                on_prev, prev_tt = None, None
                carried = []  # YO units deferred from a filler-rich tile to
                              # the filler-poor last tile
                for idx, tt in enumerate(TILE_ORDER):
                    if idx + 2 < NTT:
                        load_xq(TILE_ORDER[idx + 2])
                    if idx + 1 < NTT:
                        qt_next, q_units = make_q_units(
                            TILE_ORDER[idx + 1], qpool=psy if idx == 0 else None)
                    else:
                        qt_next, q_units = None, []
                    yo_units = (make_yo_units(prev_tt, on_prev,
                                              alt_copy=(idx == NTT - 1),
                                              tail_act=(idx == NTT - 1))
                                if on_prev is not None else [])
                    # order: Q sweep 0, a few YO groups (covers the DVE
                    # latency of the sweep-0 psum->qt copies before sweep 1's
                    # bank reuse), Q sweep 1, remaining YO
                    fillers = (q_units[:NE] + yo_units[:2 * NDIAG]
                               + q_units[NE:] + yo_units[2 * NDIAG:])
                    on_prev = attention(tt, qt_cur, fillers,
                                        lend_q_psum=(idx == NTT - 1))
                    qt_cur, prev_tt = qt_next, tt
                # drain: out projection for the last processed tile
                for u in make_yo_units(prev_tt, on_prev, alt_copy=True,
                                       split_last=True):
                    u()
            xqp_cm.__exit__(None, None, None)

    nc.compile()
    return nc


_NC_CACHE = {}


def _get_nc(key=(T, S, E, HL)):
    if key not in _NC_CACHE:
        _NC_CACHE[key] = build_nc(T=key[0], S=key[1], E=key[2], HL=key[3])
    return _NC_CACHE[key]


def _bf16(x):
    return np.asarray(x, dtype=np.float32).astype(ml_dtypes.bfloat16)


def _tile_pne(xT):
    """[E, cols] -> [P, NE, cols] so one DMA loads several 128-row e-chunks."""
    E_, cols = xT.shape
    return np.ascontiguousarray(
        xT.reshape(E_ // 128, 128, cols).transpose(1, 0, 2))


def kernel(inputs_q, inputs_kv, Wq, Wk, Wv, Wo):
    inputs_q = np.asarray(inputs_q, dtype=np.float32)
    inputs_kv = np.asarray(inputs_kv, dtype=np.float32)
    Wq = np.asarray(Wq, dtype=np.float32)
    Wk = np.asarray(Wk, dtype=np.float32)
    Wv = np.asarray(Wv, dtype=np.float32)
    Wo = np.asarray(Wo, dtype=np.float32)

    nc = _get_nc()

    # shard: core c -> batch c//4, heads [ (c%4)*4, +4 )
    xqT_b = [_tile_pne(_bf16(inputs_q[b].T)) for b in range(B)]
    xkvT_b = [_tile_pne(_bf16(inputs_kv[b].T)) for b in range(B)]
    in_maps = []
    for c in range(N_CORES):
        b, g = divmod(c, N_CORES // B)
        h0 = g * HL
        in_maps.append({
            "xqT": xqT_b[b],
            "xkvT": xkvT_b[b],
            "wq": _tile_pne(_bf16(Wq[:, h0:h0 + HL, :].reshape(E, HL * D))),
            "wk": _tile_pne(_bf16(Wk[:, h0:h0 + HL, :].reshape(E, HL * D))),
            "wv": _tile_pne(_bf16(Wv[:, h0:h0 + HL, :].reshape(E, HL * D))),
            "wo": _bf16(np.ascontiguousarray(Wo[h0:h0 + HL])),
        })

    res = bass_utils.run_bass_kernel_spmd(nc, in_maps, core_ids=list(range(N_CORES)))

    out = np.zeros((B, T, E), dtype=np.float32)
    for c in range(N_CORES):
        b = c // (N_CORES // B)
        out[b] += res.results[c]["y"]
    return out
